# revision 7
# baseline (speedup 1.0000x reference)
"""Positional-encoding add kernel for Trainium2 (8 NeuronCores, SPMD).

out = X + PE, X [4, 4096, 2048] f32, PE interleaved sin/cos table.

Sharding: sequence split 8 ways -> 512 positions/core; per-core shard
viewed as [2048, 2048] (row = b*512 + s_local), streamed as 8 tiles of
[128, 4096] (2 MiB; partition p holds rows 2p, 2p+1 of a 256-row block).

PE is computed ON-CHIP (no 4 MiB table load): for tile parity k and
half r_idx, position = c*512 + k*256 + 2p + r_idx, and
PE[pos, jj] = sin(pos*invfreq2[jj] + phase[jj]) with
invfreq2[jj] = 1/10000^(2*(jj//2)/2048), phase = 0 / pi/2 interleaved.
angle -> k=round(angle/2pi) (magic-number) -> Cody-Waite reduce -> ACT Sin.
Only [1,2048] invfreq/phase rows (broadcast-DMA'd) + [128,4] POS go to HBM.

Streaming is PHASED per rep: all 8 loads issue on the sync HWDGE ring
before any store; stores follow on the same ring so each SDMA engine
drains a pure-read phase then a pure-write phase (mixed R/W at the HBM
controller costs ~8%). Adds run on DVE, overlapped under DMA.
"""

import os

import numpy as np

B, S, D = 4, 4096, 2048
N_CORES = 8
S_SHARD = S // N_CORES          # 512 positions per core
ROWS = B * S_SHARD              # 2048 rows per core
P = 128                         # SBUF partitions
R = 2                           # rows per partition -> 2 MiB tiles
FREE = R * D                    # 4096
N_TILES = ROWS // (P * R)       # 8
N_PE = S_SHARD // (P * R)       # 2
X_BUFS = 7

PI = float(np.float32(np.pi))
INV2PI = float(np.float32(1.0 / (2 * np.pi)))
MAGIC = 12582912.0              # 1.5 * 2^23: f32 round-to-int bias
_TWO_PI = 2 * np.pi
_C1 = float(np.float32(np.ldexp(np.round(np.ldexp(_TWO_PI, 11)), -11)))
_C2 = float(np.float32(np.ldexp(np.round(np.ldexp(_TWO_PI - _C1, 35)), -35)))
_C3 = float(np.float32(_TWO_PI - _C1 - _C2))

_cached_nc = None
LAST_RESULT = None              # BassKernelResults of the last run (for test.py)


def _build_nc(repeat: int = 1):
    import concourse.bacc as bacc
    import concourse.mybir as mybir
    from concourse.alu_op_type import AluOpType
    from concourse.tile import TileContext

    f32 = mybir.dt.float32
    nc = bacc.Bacc(None, target_bir_lowering=False, debug=False)
    x = nc.dram_tensor("X", [ROWS, D], f32, kind="ExternalInput")
    pos = nc.dram_tensor("POS", [P, 2 * N_PE], f32, kind="ExternalInput")
    consts = nc.dram_tensor("CONSTS", [2, D], f32, kind="ExternalInput")
    out = nc.dram_tensor("OUT", [ROWS, D], f32, kind="ExternalOutput")

    xv = x.rearrange("(t p r) d -> t p (r d)", t=N_TILES, p=P, r=R)
    ov = out.rearrange("(t p r) d -> t p (r d)", t=N_TILES, p=P, r=R)

    with TileContext(nc) as tc:
        with (
            tc.tile_pool(name="pe", bufs=1) as pe_pool,
            tc.tile_pool(name="xs", bufs=1) as xs_pool,
            tc.tile_pool(name="ps", bufs=1, space="PSUM") as ps_pool,
        ):
            # --- prologue: compute the PE tiles on-chip (outside rep loop)
            post = pe_pool.tile([P, 2 * N_PE], f32)
            nc.sync.dma_start(out=post, in_=pos[:, :])
            # Broadcast the [1, D] const rows across partitions with a
            # ones-column outer product on the (otherwise idle) tensor
            # engine — only 16 KiB of HBM instead of 2 MiB of replicated
            # row reads.
            crow0 = pe_pool.tile([1, D], f32)
            nc.sync.dma_start(out=crow0, in_=consts[0:1, :])
            crow1 = pe_pool.tile([1, D], f32)
            nc.sync.dma_start(out=crow1, in_=consts[1:2, :])
            ones = pe_pool.tile([1, P], f32)
            nc.vector.memset(ones, 1.0)
            iff = pe_pool.tile([P, D], f32)
            phs = pe_pool.tile([P, D], f32)
            for crow, dst in ((crow0, iff), (crow1, phs)):
                pt = ps_pool.tile([P, D], f32, name="bc")
                for c0 in range(0, D, 512):
                    nc.tensor.matmul(
                        pt[:, c0 : c0 + 512],
                        ones[0:1, :],
                        crow[0:1, c0 : c0 + 512],
                        start=True,
                        stop=True,
                    )
                nc.scalar.copy(out=dst, in_=pt)

            pe_ts = [pe_pool.tile([P, FREE], f32, name=f"pe{k}") for k in range(N_PE)]
            for k in range(N_PE):
                for r_idx in range(R):
                    ang = pe_pool.tile([P, D], f32, bufs=2)
                    nc.vector.scalar_tensor_tensor(
                        out=ang, in0=iff, scalar=post[:, R * k + r_idx : R * k + r_idx + 1],
                        in1=phs, op0=AluOpType.mult, op1=AluOpType.add,
                    )
                    ym = pe_pool.tile([P, D], f32, bufs=1)
                    nc.vector.tensor_scalar(
                        out=ym, in0=ang, scalar1=INV2PI, scalar2=MAGIC,
                        op0=AluOpType.mult, op1=AluOpType.add,
                    )
                    nc.vector.tensor_scalar(
                        out=ym, in0=ym, scalar1=MAGIC, scalar2=None,
                        op0=AluOpType.subtract,
                    )
                    nc.vector.cody_waite_cascade(
                        out=ang, x=ang, k=ym, c1=_C1, c2=_C2, c3=_C3
                    )
                    nc.scalar.activation(
                        out=pe_ts[k][:, r_idx * D : (r_idx + 1) * D], in_=ang,
                        func=mybir.ActivationFunctionType.Sin,
                    )

            # --- main loop: phased load/store streaming
            for _rep in range(repeat):
                tiles = []
                for t in range(N_TILES):
                    xt = xs_pool.tile([P, FREE], f32, name="xt", bufs=X_BUFS)
                    nc.sync.dma_start(out=xt, in_=xv[t])
                    tiles.append(xt)
                for t in range(N_TILES):
                    xt = tiles[t]
                    nc.vector.tensor_add(out=xt, in0=xt, in1=pe_ts[t % N_PE])
                    nc.sync.dma_start(out=ov[t], in_=xt)
    nc.finalize()
    return nc


def _host_inputs(core: int):
    """POS [128, 4] and CONSTS [2, 2048] f32 for `core`."""
    i = np.arange(D // 2, dtype=np.float32)
    denom = np.power(
        np.float32(10000.0), (np.float32(2.0) * i) / np.float32(D), dtype=np.float32
    )
    invfreq = (np.float32(1.0) / denom).astype(np.float32)
    invfreq2 = np.repeat(invfreq, 2)                       # [2048]
    phase = np.tile(np.array([0.0, np.pi / 2], np.float32), D // 2)
    consts = np.stack([invfreq2, phase.astype(np.float32)])
    posm = np.empty((P, 2 * N_PE), np.float32)
    for k in range(N_PE):
        for r_idx in range(R):
            posm[:, R * k + r_idx] = (
                core * S_SHARD + k * P * R + R * np.arange(P) + r_idx
            )
    return posm.astype(np.float32), consts.astype(np.float32)


def kernel(X: np.ndarray) -> np.ndarray:
    global _cached_nc, LAST_RESULT
    from concourse.bass_utils import run_bass_kernel_spmd

    X = np.asarray(X)
    assert X.shape == (B, S, D), X.shape
    X = np.ascontiguousarray(X, dtype=np.float32)

    if _cached_nc is None:
        _cached_nc = _build_nc()
    nc = _cached_nc

    in_maps = []
    for c in range(N_CORES):
        xs = np.ascontiguousarray(X[:, c * S_SHARD : (c + 1) * S_SHARD, :]).reshape(
            ROWS, D
        )
        posm, consts = _host_inputs(c)
        in_maps.append({"X": xs, "POS": posm, "CONSTS": consts})

    trace = bool(int(os.environ.get("KERNEL_TRACE", "0")))
    try:
        res = run_bass_kernel_spmd(
            nc, in_maps, core_ids=list(range(N_CORES)), trace=trace
        )
    except ModuleNotFoundError:
        # Tracing hooks unavailable in this image (no antenv) — run untraced.
        os.environ["BASS_NEVER_TRACE"] = "1"
        res = run_bass_kernel_spmd(
            nc, in_maps, core_ids=list(range(N_CORES)), trace=False
        )
    LAST_RESULT = res

    out = np.empty((B, S, D), dtype=np.float32)
    for c in range(N_CORES):
        out[:, c * S_SHARD : (c + 1) * S_SHARD, :] = res.results[c]["OUT"].reshape(
            B, S_SHARD, D
        )
    return out


# revision 8
# speedup vs baseline: 1.0317x; 1.0317x over previous
"""Positional-encoding add kernel for Trainium2 (8 NeuronCores, SPMD).

out = X + PE, X [4, 4096, 2048] f32, PE interleaved sin/cos table.

Sharding: sequence split 8 ways -> 512 positions/core; per-core shard
viewed as [2048, 2048] (row = b*512 + s_local), streamed as 8 tiles of
[128, 4096] (2 MiB; partition p holds rows 2p, 2p+1 of a 256-row block).

PE is computed ON-CHIP (no 4 MiB table load): for tile parity k and
half r_idx, position = c*512 + k*256 + 2p + r_idx, and
PE[pos, jj] = sin(pos*invfreq2[jj] + phase[jj]) with
invfreq2[jj] = 1/10000^(2*(jj//2)/2048), phase = 0 / pi/2 interleaved.
angle -> k=round(angle/2pi) (magic-number) -> Cody-Waite reduce -> ACT Sin.
Only [1,2048] invfreq/phase rows (broadcast-DMA'd) + [128,4] POS go to HBM.

Streaming is PHASED per rep: all 8 loads issue on the sync HWDGE ring
before any store; stores follow on the same ring so each SDMA engine
drains a pure-read phase then a pure-write phase (mixed R/W at the HBM
controller costs ~8%). Adds run on DVE, overlapped under DMA.
"""

import os

import numpy as np

B, S, D = 4, 4096, 2048
N_CORES = 8
S_SHARD = S // N_CORES          # 512 positions per core
ROWS = B * S_SHARD              # 2048 rows per core
P = 128                         # SBUF partitions
R = 2                           # rows per partition -> 2 MiB tiles
FREE = R * D                    # 4096
N_TILES = ROWS // (P * R)       # 8
N_PE = S_SHARD // (P * R)       # 2
X_BUFS = 7

PI = float(np.float32(np.pi))
INV2PI = float(np.float32(1.0 / (2 * np.pi)))
MAGIC = 12582912.0              # 1.5 * 2^23: f32 round-to-int bias
_TWO_PI = 2 * np.pi
_C1 = float(np.float32(np.ldexp(np.round(np.ldexp(_TWO_PI, 11)), -11)))
_C2 = float(np.float32(np.ldexp(np.round(np.ldexp(_TWO_PI - _C1, 35)), -35)))
_C3 = float(np.float32(_TWO_PI - _C1 - _C2))

_cached_nc = None
LAST_RESULT = None              # BassKernelResults of the last run (for test.py)


def _build_nc(repeat: int = 1):
    import concourse.bacc as bacc
    import concourse.mybir as mybir
    from concourse.alu_op_type import AluOpType
    from concourse.tile import TileContext

    f32 = mybir.dt.float32
    nc = bacc.Bacc(None, target_bir_lowering=False, debug=False)
    x = nc.dram_tensor("X", [ROWS, D], f32, kind="ExternalInput")
    pos = nc.dram_tensor("POS", [P, 2 * N_PE], f32, kind="ExternalInput")
    consts = nc.dram_tensor("CONSTS", [2, D], f32, kind="ExternalInput")
    out = nc.dram_tensor("OUT", [ROWS, D], f32, kind="ExternalOutput")

    xv = x.rearrange("(t p r) d -> t p (r d)", t=N_TILES, p=P, r=R)
    ov = out.rearrange("(t p r) d -> t p (r d)", t=N_TILES, p=P, r=R)

    with TileContext(nc) as tc:
        with (
            tc.tile_pool(name="pe", bufs=1) as pe_pool,
            tc.tile_pool(name="xs", bufs=1) as xs_pool,
            tc.tile_pool(name="ps", bufs=1, space="PSUM") as ps_pool,
        ):
            # --- prologue: compute the PE tiles on-chip (outside rep loop)
            # Prologue inputs ride the gpsimd SWDGE ring so the sync ring's
            # FIFO starts the bulk X loads at t=0.
            post = pe_pool.tile([P, 2 * N_PE], f32)
            nc.gpsimd.dma_start(out=post, in_=pos[:, :])
            # Broadcast the [1, D] const rows across partitions with a
            # ones-column outer product on the (otherwise idle) tensor
            # engine — only 16 KiB of HBM instead of 2 MiB of replicated
            # row reads.
            crow0 = pe_pool.tile([1, D], f32)
            nc.gpsimd.dma_start(out=crow0, in_=consts[0:1, :])
            crow1 = pe_pool.tile([1, D], f32)
            nc.gpsimd.dma_start(out=crow1, in_=consts[1:2, :])
            ones = pe_pool.tile([1, P], f32)
            nc.vector.memset(ones, 1.0)
            iff = pe_pool.tile([P, D], f32)
            phs = pe_pool.tile([P, D], f32)
            for crow, dst in ((crow0, iff), (crow1, phs)):
                pt = ps_pool.tile([P, D], f32, name="bc")
                for c0 in range(0, D, 512):
                    nc.tensor.matmul(
                        pt[:, c0 : c0 + 512],
                        ones[0:1, :],
                        crow[0:1, c0 : c0 + 512],
                        start=True,
                        stop=True,
                    )
                nc.scalar.copy(out=dst, in_=pt)

            pe_ts = [pe_pool.tile([P, FREE], f32, name=f"pe{k}") for k in range(N_PE)]
            for k in range(N_PE):
                for r_idx in range(R):
                    ang = pe_pool.tile([P, D], f32, bufs=2)
                    nc.vector.scalar_tensor_tensor(
                        out=ang, in0=iff, scalar=post[:, R * k + r_idx : R * k + r_idx + 1],
                        in1=phs, op0=AluOpType.mult, op1=AluOpType.add,
                    )
                    ym = pe_pool.tile([P, D], f32, bufs=1)
                    nc.vector.tensor_scalar(
                        out=ym, in0=ang, scalar1=INV2PI, scalar2=MAGIC,
                        op0=AluOpType.mult, op1=AluOpType.add,
                    )
                    nc.vector.tensor_scalar(
                        out=ym, in0=ym, scalar1=MAGIC, scalar2=None,
                        op0=AluOpType.subtract,
                    )
                    nc.vector.cody_waite_cascade(
                        out=ang, x=ang, k=ym, c1=_C1, c2=_C2, c3=_C3
                    )
                    nc.scalar.activation(
                        out=pe_ts[k][:, r_idx * D : (r_idx + 1) * D], in_=ang,
                        func=mybir.ActivationFunctionType.Sin,
                    )

            # --- main loop: phased load/store streaming
            for _rep in range(repeat):
                tiles = []
                for t in range(N_TILES):
                    xt = xs_pool.tile([P, FREE], f32, name="xt", bufs=X_BUFS)
                    nc.sync.dma_start(out=xt, in_=xv[t])
                    tiles.append(xt)
                for t in range(N_TILES):
                    xt = tiles[t]
                    nc.vector.tensor_add(out=xt, in0=xt, in1=pe_ts[t % N_PE])
                    nc.sync.dma_start(out=ov[t], in_=xt)
    nc.finalize()
    return nc


def _host_inputs(core: int):
    """POS [128, 4] and CONSTS [2, 2048] f32 for `core`."""
    i = np.arange(D // 2, dtype=np.float32)
    denom = np.power(
        np.float32(10000.0), (np.float32(2.0) * i) / np.float32(D), dtype=np.float32
    )
    invfreq = (np.float32(1.0) / denom).astype(np.float32)
    invfreq2 = np.repeat(invfreq, 2)                       # [2048]
    phase = np.tile(np.array([0.0, np.pi / 2], np.float32), D // 2)
    consts = np.stack([invfreq2, phase.astype(np.float32)])
    posm = np.empty((P, 2 * N_PE), np.float32)
    for k in range(N_PE):
        for r_idx in range(R):
            posm[:, R * k + r_idx] = (
                core * S_SHARD + k * P * R + R * np.arange(P) + r_idx
            )
    return posm.astype(np.float32), consts.astype(np.float32)


def kernel(X: np.ndarray) -> np.ndarray:
    global _cached_nc, LAST_RESULT
    from concourse.bass_utils import run_bass_kernel_spmd

    X = np.asarray(X)
    assert X.shape == (B, S, D), X.shape
    X = np.ascontiguousarray(X, dtype=np.float32)

    if _cached_nc is None:
        _cached_nc = _build_nc()
    nc = _cached_nc

    in_maps = []
    for c in range(N_CORES):
        xs = np.ascontiguousarray(X[:, c * S_SHARD : (c + 1) * S_SHARD, :]).reshape(
            ROWS, D
        )
        posm, consts = _host_inputs(c)
        in_maps.append({"X": xs, "POS": posm, "CONSTS": consts})

    trace = bool(int(os.environ.get("KERNEL_TRACE", "0")))
    try:
        res = run_bass_kernel_spmd(
            nc, in_maps, core_ids=list(range(N_CORES)), trace=trace
        )
    except ModuleNotFoundError:
        # Tracing hooks unavailable in this image (no antenv) — run untraced.
        os.environ["BASS_NEVER_TRACE"] = "1"
        res = run_bass_kernel_spmd(
            nc, in_maps, core_ids=list(range(N_CORES)), trace=False
        )
    LAST_RESULT = res

    out = np.empty((B, S, D), dtype=np.float32)
    for c in range(N_CORES):
        out[:, c * S_SHARD : (c + 1) * S_SHARD, :] = res.results[c]["OUT"].reshape(
            B, S_SHARD, D
        )
    return out
